# revision 77
# baseline (speedup 1.0000x reference)
# DeepSet Trainium2 kernel.
#
# Strategy: events are sorted by jet-count n (2..10) on the host and
# round-robin sharded across 8 cores into per-group slots of capacity cap_g
# (multiple of 8, exact-packed). Within a group every event has exactly n=g
# valid jets, so all masks, pair structures and aggregation counts are
# compile-time constants.
#
# Math folding (host, O(params)):
#   every Dense+BN+relu block becomes relu(h @ W' + b') with W', b' folded.
#   MLP2 layer 1 uses the z-trick: y1 = relu(z_i + z_j + t) with z = x @ Wz'.
#   t is folded into the y1 relu pass (tensor_scalar add-bias + max0).
#
# Device layout: feature-major [H=128 partitions, columns = slice*cap + b]
# per group, all activations bf16 (PE: 1 col/cycle), PSUM f32.
# The device computes ONLY the MLP chains:
#   jets:  x1 = relu(W1 jt + b1); x2 = relu(W2 x1 + b2); x = relu(W3 x2 + b3)
#          z = Wz x (plain copy evac)
#   pairs: y1 = relu(z_i + z_j + t) (DVE broadcast-add + tensor_scalar 4x)
#          y2 = relu(W4 y1 + b4); y3 = relu(W5 y2 + b5)
# and streams x and y3 (bf16) to DRAM. All aggregations (sum/max/mean/var
# over jets and pairs) happen on the HOST in f32 — the DMA engines were
# ~10% busy while Vector/Scalar were the kernel bottleneck, so shipping
# raw activations beats computing reductions on-device.
#
# Engine split (measured rates, ns/col of 128 rows):
#   Scalar  (~1.05): x1/x2/x/z PSUM evacs, y2 evac, y3 evac (2 of 3)
#   Vector  (~0.65 TT / 0.26 TSP-bf16 / 1.05 PSUM): y1 add, y1 relu,
#           y3 evac (1 of 3)
#   PE:     all matmuls (bf16 1 col/cycle, dual-buffered weight loads)
#   GPSIMD: unused — no PSUM access, no max opcode, slow tensor_scalar,
#           and its tensor_tensor traffic degrades Vector ~25% (measured).
#
# The next group's jets layer-chunks are emitted interleaved into the
# current group's pair-chunk loop (groups in descending size order), so
# PE/Scalar advance the next group while Vector drains the current one.
from contextlib import ExitStack

import numpy as np

import concourse.bass as bass
import concourse.bacc as bacc
import concourse.tile as tile
import concourse.mybir as mybir

f32 = mybir.dt.float32
bf16 = mybir.dt.bfloat16
AF = mybir.ActivationFunctionType
ALU = mybir.AluOpType

H = 128
FJ = 16


def pairs_of(g):
    return [(i, j) for i in range(g) for j in range(i + 1, g)]


# y3 PSUM-evac engine pattern (s=Scalar, v=Vector), tuned from traces.
Y3_PAT = ("s", "v")


def build_program(groups):
    """groups: list of (g, cap) with cap a multiple of 8, cap <= 512."""
    JC = sum(g * cap for g, cap in groups)
    YC = sum((g * (g - 1) // 2) * cap for g, cap in groups)

    nc = bacc.Bacc("TRN2", target_bir_lowering=False, debug=False)

    jets_d = nc.dram_tensor("jets", [FJ, JC], bf16, kind="ExternalInput")
    w1_d = nc.dram_tensor("w1", [FJ, H], bf16, kind="ExternalInput")
    w2_d = nc.dram_tensor("w2", [H, H], bf16, kind="ExternalInput")
    w3_d = nc.dram_tensor("w3", [H, H], bf16, kind="ExternalInput")
    wz_d = nc.dram_tensor("wz", [H, H], bf16, kind="ExternalInput")
    w4_d = nc.dram_tensor("w4", [H, H], bf16, kind="ExternalInput")
    w5_d = nc.dram_tensor("w5", [H, H], bf16, kind="ExternalInput")
    # bias vector cols: 0..5 = b1, b2, b3, t(=bz), b4, b5
    bv_d = nc.dram_tensor("bvec", [H, 8], f32, kind="ExternalInput")
    xout_d = nc.dram_tensor("xout", [H, JC], bf16, kind="ExternalOutput")
    yout_d = nc.dram_tensor("yout", [H, YC], bf16, kind="ExternalOutput")

    with tile.TileContext(nc) as tc, ExitStack() as ctx:
        consts = ctx.enter_context(tc.tile_pool(name="consts", bufs=1))
        big = ctx.enter_context(tc.tile_pool(name="big", bufs=1))
        scr = ctx.enter_context(tc.tile_pool(name="scr", bufs=3))
        mm = ctx.enter_context(tc.tile_pool(name="mm", bufs=4, space="PSUM"))

        def const_tile(name, dram, shape, dt):
            t = consts.tile(shape, dt, tag=name)
            nc.sync.dma_start(t[:], dram.ap())
            return t

        # Load only what the first L1 chunk needs (w1 + biases) before the
        # jets input stream; later-phase weights queue behind it so the
        # first matmul starts ~4us earlier.
        w1t = const_tile("w1", w1_d, [FJ, H], bf16)
        bv = const_tile("bv", bv_d, [H, 8], f32)

        # ---- phase 1: jets MLP over the whole [*, JC] width, no group
        # boundaries. Layer-major: every chunk of a layer only depends on
        # the same chunk of the previous layer (emitted a full layer
        # earlier), so all three engines pipeline freely.
        jt = big.tile([FJ, JC], bf16, tag="jt")
        nc.sync.dma_start(jt[:, 0:1024], jets_d.ap()[:, 0:1024])
        for c0 in range(1024, JC, 2048):
            w = min(2048, JC - c0)
            nc.sync.dma_start(jt[:, c0 : c0 + w],
                              jets_d.ap()[:, c0 : c0 + w])

        w2t = const_tile("w2", w2_d, [H, H], bf16)
        w3t = const_tile("w3", w3_d, [H, H], bf16)
        wzt = const_tile("wz", wz_d, [H, H], bf16)
        w4t = const_tile("w4", w4_d, [H, H], bf16)
        w5t = const_tile("w5", w5_d, [H, H], bf16)

        def r3(ap, k):
            return ap.rearrange("p (k c) -> p k c", k=k)
        x1 = big.tile([H, JC], bf16, tag="x1")
        x2 = big.tile([H, JC], bf16, tag="x2")
        x = big.tile([H, JC], bf16, tag="x")
        z = big.tile([H, JC], bf16, tag="z")
        plan = [(x1, w1t, jt, 0), (x2, w2t, x1, 1),
                (x, w3t, x2, 2), (z, wzt, x, None)]
        for li, (dst, wt, src, bias_col) in enumerate(plan):
            for ci, c0 in enumerate(range(0, JC, 1024)):
                w = min(1024, JC - c0)
                ps = mm.tile([H, 1024], f32, tag="mm")
                for s0 in range(0, w, 512):
                    sw = min(512, w - s0)
                    nc.tensor.matmul(ps[:, s0 : s0 + sw], wt[:],
                                     src[:, c0 + s0 : c0 + s0 + sw],
                                     start=True, stop=True)
                # Split each layer's evacs between Vector (front half) and
                # Scalar (back half): layers are sequential, so a
                # per-layer split would idle one engine per layer; V gets
                # the front so it finishes early and rolls into y1 (whose
                # first chunks need the first z columns).
                nchunks = -(-JC // 1024)
                on_v = ci < nchunks * 6 // 13
                if bias_col is None:
                    if on_v:
                        nc.vector.tensor_copy(dst[:, c0 : c0 + w],
                                              ps[:, :w])
                    else:
                        nc.scalar.copy(dst[:, c0 : c0 + w], ps[:, :w])
                elif on_v:
                    nc.vector.tensor_scalar(
                        dst[:, c0 : c0 + w], ps[:, :w],
                        bv[:, bias_col : bias_col + 1], 0.0,
                        ALU.add, ALU.max)
                else:
                    nc.scalar.activation(
                        dst[:, c0 : c0 + w], ps[:, :w], AF.Relu,
                        bias=bv[:, bias_col : bias_col + 1])
                if li == 2:
                    nc.sync.dma_start(xout_d.ap()[:, c0 : c0 + w],
                                      dst[:, c0 : c0 + w])

        # ---- phase 2: all pair chunks stream through y1 -> y2 -> y3 ->
        # DRAM. Every chunk's z is ready, so chunks are fully independent
        # and pipeline across V (y1), PE (matmuls), S/V (evacs), DMA.
        # y3 matmuls run one chunk behind y2 (software pipeline) so PE
        # never waits on the y2 evac of the chunk it just produced.
        yout_off = 0
        y3_i = [0]
        pend = [None]  # (y2_tile, w, yout_off)

        def emit_y3(y2, w, yoff):
            y3 = scr.tile([H, 4096], bf16, tag="y3")
            for n0 in range(0, w, 1024):
                cw = min(1024, w - n0)
                ps = mm.tile([H, 1024], f32, tag="mm")
                for s0 in range(0, cw, 512):
                    sw = min(512, cw - s0)
                    nc.tensor.matmul(ps[:, s0 : s0 + sw], w5t[:],
                                     y2[:, n0 + s0 : n0 + s0 + sw],
                                     start=True, stop=True)
                dst = y3[:, n0 : n0 + cw]
                if Y3_PAT[y3_i[0] % len(Y3_PAT)] == "v":
                    nc.vector.tensor_scalar(dst, ps[:, :cw], bv[:, 5:6],
                                            0.0, ALU.add, ALU.max)
                else:
                    nc.scalar.activation(dst, ps[:, :cw], AF.Relu,
                                         bias=bv[:, 5:6])
                y3_i[0] += 1
            nc.sync.dma_start(yout_d.ap()[:, yoff : yoff + w], y3[:, :w])

        jets_off = 0
        for g, cap in groups:
            assert cap % 8 == 0 and cap <= 512
            prs = pairs_of(g)
            PG = len(prs)
            SC = max(1, 4096 // cap)
            for p0 in range(0, PG, SC):
                k = min(SC, PG - p0)
                w = k * cap
                y1 = scr.tile([H, 4096], bf16, tag="y1")
                s = 0
                while s < k:
                    i = prs[p0 + s][0]
                    r = 1
                    while s + r < k and prs[p0 + s + r][0] == i:
                        r += 1
                    j0 = prs[p0 + s][1]
                    zb = jets_off
                    nc.vector.tensor_tensor(
                        r3(y1[:, s * cap : (s + r) * cap], r),
                        r3(z[:, zb + i * cap : zb + (i + 1) * cap],
                           1).broadcast_to([H, r, cap]),
                        r3(z[:, zb + j0 * cap : zb + (j0 + r) * cap], r),
                        ALU.add)
                    s += r
                nc.vector.tensor_scalar(y1[:, :w], y1[:, :w], bv[:, 3:4],
                                        0.0, ALU.add, ALU.max)
                y2 = scr.tile([H, 4096], bf16, tag="y2")
                for n0 in range(0, w, 1024):
                    cw = min(1024, w - n0)
                    ps = mm.tile([H, 1024], f32, tag="mm")
                    for s0 in range(0, cw, 512):
                        sw = min(512, cw - s0)
                        nc.tensor.matmul(ps[:, s0 : s0 + sw], w4t[:],
                                         y1[:, n0 + s0 : n0 + s0 + sw],
                                         start=True, stop=True)
                    nc.scalar.activation(y2[:, n0 : n0 + cw], ps[:, :cw],
                                         AF.Relu, bias=bv[:, 4:5])
                if pend[0] is not None:
                    emit_y3(*pend[0])
                pend[0] = (y2, w, yout_off)
                yout_off += w
            jets_off += g * cap
        emit_y3(*pend[0])

    nc.compile()
    return nc


# ---------------- host-side math ----------------

BN_EPS = 1e-3


def fold_params(inp):
    """Fold normalization + BN into per-layer (W, b). All numpy fp32."""
    mean_j = np.asarray(inp["mean_jets"], np.float32)
    std_j = np.asarray(inp["std_jets"], np.float32)
    w1f = np.asarray(inp["w1_first"], np.float32)
    w1r = np.asarray(inp["w1_rest"], np.float32)
    bn1 = np.asarray(inp["bn1"], np.float32)  # [3,4,H]: gamma, beta, mean, var
    w2f = np.asarray(inp["w2_first"], np.float32)
    w2r = np.asarray(inp["w2_rest"], np.float32)
    bn2 = np.asarray(inp["bn2"], np.float32)

    def bn_sb(row):
        gm, bt, mu, vv = row[0], row[1], row[2], row[3]
        s = gm / np.sqrt(vv + BN_EPS)
        return s.astype(np.float32), (bt - mu * s).astype(np.float32)

    s11, t11 = bn_sb(bn1[0]); s12, t12 = bn_sb(bn1[1]); s13, t13 = bn_sb(bn1[2])
    s21, t21 = bn_sb(bn2[0]); s22, t22 = bn_sb(bn2[1]); s23, t23 = bn_sb(bn2[2])

    A = w1f / std_j[:, None]
    c = -(mean_j / std_j) @ w1f
    return dict(
        W1=A * s11[None, :], b1=c * s11 + t11,
        W2=w1r[0] * s12[None, :], b2=t12,
        W3=w1r[1] * s13[None, :], b3=t13,
        Wz=w2f * s21[None, :], bz=t21,
        W4=w2r[0] * s22[None, :], b4=t22,
        W5=w2r[1] * s23[None, :], b5=t23,
    )


# ---------------- full kernel entry point ----------------

N_CORES = 8

_cache = {}
_TRACE = [False]
_LAST_RESULT = [None]


def _get_program(groups_key):
    if groups_key not in _cache:
        _cache[groups_key] = build_program(list(groups_key))
    return _cache[groups_key]


def _np_dt(dt):
    return mybir.dt.np(dt)


def _plan(n):
    """Returns (groups, slots): groups = [(g, cap)], slots[c][gi] =
    (padded index array, real count) for core c, group gi."""
    gs = []
    idx_by_g = {}
    for g in range(2, 11):
        idx = np.nonzero(n == g)[0]
        if len(idx):
            gs.append(g)
            idx_by_g[g] = idx
    stray = np.nonzero((n < 2) | (n > 10))[0]
    if len(stray):
        if not gs:
            gs.append(2)
            idx_by_g[2] = stray
        else:
            idx_by_g[gs[-1]] = np.concatenate([idx_by_g[gs[-1]], stray])
    # Descending size order: each group's jets chain is emitted inside
    # the previous (bigger) group's pair phase, so it pipelines fully.
    gs = sorted(gs, key=lambda g: -g)
    groups = []
    slots = [[] for _ in range(N_CORES)]
    for g in gs:
        idx = idx_by_g[g]
        per_core = [idx[c::N_CORES] for c in range(N_CORES)]
        mx = max(len(p) for p in per_core)
        cap = max(8, ((mx + 7) // 8) * 8)
        groups.append((g, cap))
        fill = idx[0]
        for c in range(N_CORES):
            p = per_core[c]
            pad = np.full(cap, p[0] if len(p) else fill, dtype=np.int64)
            pad[: len(p)] = p
            slots[c].append((pad, len(p)))
    return groups, slots


def _pack_jets(jets, groups, slots_c):
    cols = []
    for (g, cap), (ids, _cnt) in zip(groups, slots_c):
        ev = jets[ids][:, :g, :]  # [cap, g, 16]
        cols.append(np.ascontiguousarray(ev.transpose(2, 1, 0)).reshape(
            FJ, g * cap))
    return np.concatenate(cols, axis=1).astype(_np_dt(bf16), copy=False)


def kernel(**inputs):
    from concourse.bass_utils import run_bass_kernel_spmd

    jets = np.asarray(inputs["inputs_jets"], dtype=np.float32)
    B = jets.shape[0]
    mask = (jets != 0.0).any(-1)
    n = mask.sum(-1).astype(np.int64)
    # compact valid jets to the front (no-op for the standard generator)
    if not np.array_equal(mask, np.arange(jets.shape[1])[None, :] < n[:, None]):
        order = np.argsort(~mask, axis=1, kind="stable")
        jets = np.take_along_axis(jets, order[:, :, None], axis=1)

    P = fold_params(inputs)
    groups, slots = _plan(n)
    nc = _get_program(tuple(groups))

    bvec = np.zeros((H, 8), np.float32)
    for i, k in enumerate(["b1", "b2", "b3", "bz", "b4", "b5"]):
        bvec[:, i] = P[k]
    bnp = _np_dt(bf16)
    common = {
        "w1": P["W1"].astype(bnp), "w2": P["W2"].astype(bnp),
        "w3": P["W3"].astype(bnp), "wz": P["Wz"].astype(bnp),
        "w4": P["W4"].astype(bnp), "w5": P["W5"].astype(bnp),
        "bvec": bvec,
    }
    in_maps = []
    for c in range(N_CORES):
        m = dict(common)
        m["jets"] = _pack_jets(jets, groups, slots[c])
        in_maps.append(m)

    res = run_bass_kernel_spmd(nc, in_maps, core_ids=list(range(N_CORES)),
                               trace=_TRACE[0])
    _LAST_RESULT[0] = res

    agg_x = np.empty((B, 4 * H), np.float32)
    agg_y = np.empty((B, 4 * H), np.float32)
    for c in range(N_CORES):
        ox = np.asarray(res.results[c]["xout"])  # [H, JC] bf16
        oy = np.asarray(res.results[c]["yout"])  # [H, YC] bf16
        joff = 0
        yoff = 0
        for (g, cap), (ids, cnt) in zip(groups, slots[c]):
            PGg = g * (g - 1) // 2
            ii = ids[:cnt]
            xb = ox[:, joff : joff + g * cap].astype(np.float32)
            xb = xb.reshape(H, g, cap)[:, :, :cnt]
            sx = xb.sum(1).T
            mx = xb.max(1).T
            qx = (xb * xb).sum(1).T
            mean_x = sx / g
            agg_x[ii] = np.concatenate(
                [sx, mx, mean_x, qx / g - mean_x * mean_x], axis=1)
            yb = oy[:, yoff : yoff + PGg * cap].astype(np.float32)
            yb = yb.reshape(H, PGg, cap)[:, :, :cnt]
            sy = yb.sum(1).T
            my = yb.max(1).T
            qy = (yb * yb).sum(1).T
            mean_y = sy / PGg
            agg_y[ii] = np.concatenate(
                [sy, my, mean_y, qy / PGg - mean_y * mean_y], axis=1)
            joff += g * cap
            yoff += PGg * cap
    return agg_x, agg_y


# revision 81
# speedup vs baseline: 1.1853x; 1.1853x over previous
# DeepSet Trainium2 kernel.
#
# Strategy: events are sorted by jet-count n (2..10) on the host and
# round-robin sharded across 8 cores into per-group slots of capacity cap_g
# (multiple of 8, exact-packed). Within a group every event has exactly n=g
# valid jets, so all masks, pair structures and aggregation counts are
# compile-time constants.
#
# Math folding (host, O(params)):
#   every Dense+BN+relu block becomes relu(h @ W' + b') with W', b' folded.
#   MLP2 layer 1 uses the z-trick: y1 = relu(z_i + z_j + t) with z = x @ Wz'.
#   t is folded into the y1 relu pass (tensor_scalar add-bias + max0).
#
# Device layout: feature-major [H=128 partitions, columns = slice*cap + b]
# per group, all activations bf16 (PE: 1 col/cycle), PSUM f32.
# The device computes ONLY the MLP chains:
#   jets:  x1 = relu(W1 jt + b1); x2 = relu(W2 x1 + b2); x = relu(W3 x2 + b3)
#          z = Wz x (plain copy evac)
#   pairs: y1 = relu(z_i + z_j + t) (DVE broadcast-add + tensor_scalar 4x)
#          y2 = relu(W4 y1 + b4); y3 = relu(W5 y2 + b5)
# and streams x and y3 (bf16) to DRAM. All aggregations (sum/max/mean/var
# over jets and pairs) happen on the HOST in f32 — the DMA engines were
# ~10% busy while Vector/Scalar were the kernel bottleneck, so shipping
# raw activations beats computing reductions on-device.
#
# Engine split (measured rates, ns/col of 128 rows):
#   Scalar  (~1.05): x1/x2/x/z PSUM evacs, y2 evac, y3 evac (2 of 3)
#   Vector  (~0.65 TT / 0.26 TSP-bf16 / 1.05 PSUM): y1 add, y1 relu,
#           y3 evac (1 of 3)
#   PE:     all matmuls (bf16 1 col/cycle, dual-buffered weight loads)
#   GPSIMD: unused — no PSUM access, no max opcode, slow tensor_scalar,
#           and its tensor_tensor traffic degrades Vector ~25% (measured).
#
# The next group's jets layer-chunks are emitted interleaved into the
# current group's pair-chunk loop (groups in descending size order), so
# PE/Scalar advance the next group while Vector drains the current one.
from contextlib import ExitStack

import numpy as np

import concourse.bass as bass
import concourse.bacc as bacc
import concourse.tile as tile
import concourse.mybir as mybir

f32 = mybir.dt.float32
bf16 = mybir.dt.bfloat16
AF = mybir.ActivationFunctionType
ALU = mybir.AluOpType

H = 128
FJ = 16


def pairs_of(g):
    return [(i, j) for i in range(g) for j in range(i + 1, g)]


# y3 PSUM-evac engine pattern (s=Scalar, v=Vector), tuned from traces.
Y3_PAT = ("s", "v")


def build_program(groups):
    """groups: list of (g, cap) with cap a multiple of 8, cap <= 512."""
    JC = sum(g * cap for g, cap in groups)
    YC = sum((g * (g - 1) // 2) * cap for g, cap in groups)

    nc = bacc.Bacc("TRN2", target_bir_lowering=False, debug=False)

    jets_d = nc.dram_tensor("jets", [FJ, JC], bf16, kind="ExternalInput")
    w1_d = nc.dram_tensor("w1", [FJ, H], bf16, kind="ExternalInput")
    # w2|w3|wz|w4|w5 concatenated: one DMA dispatch instead of five.
    wall_d = nc.dram_tensor("wall", [H, 5 * H], bf16, kind="ExternalInput")
    # bias vector cols: 0..5 = b1, b2, b3, t(=bz), b4, b5
    bv_d = nc.dram_tensor("bvec", [H, 8], f32, kind="ExternalInput")
    xout_d = nc.dram_tensor("xout", [H, JC], bf16, kind="ExternalOutput")
    yout_d = nc.dram_tensor("yout", [H, YC], bf16, kind="ExternalOutput")

    with tile.TileContext(nc) as tc, ExitStack() as ctx:
        consts = ctx.enter_context(tc.tile_pool(name="consts", bufs=1))
        big = ctx.enter_context(tc.tile_pool(name="big", bufs=1))
        scr = ctx.enter_context(tc.tile_pool(name="scr", bufs=3))
        mm = ctx.enter_context(tc.tile_pool(name="mm", bufs=4, space="PSUM"))

        def const_tile(name, dram, shape, dt):
            t = consts.tile(shape, dt, tag=name)
            nc.sync.dma_start(t[:], dram.ap())
            return t

        w1t = const_tile("w1", w1_d, [FJ, H], bf16)
        wall = const_tile("wall", wall_d, [H, 5 * H], bf16)
        bv = const_tile("bv", bv_d, [H, 8], f32)
        w2t = wall[:, 0 * H : 1 * H]
        w3t = wall[:, 1 * H : 2 * H]
        wzt = wall[:, 2 * H : 3 * H]
        w4t = wall[:, 3 * H : 4 * H]
        w5t = wall[:, 4 * H : 5 * H]

        def r3(ap, k):
            return ap.rearrange("p (k c) -> p k c", k=k)

        # ---- phase 1: jets MLP over the whole [*, JC] width, no group
        # boundaries. Layer-major: every chunk of a layer only depends on
        # the same chunk of the previous layer (emitted a full layer
        # earlier), so all three engines pipeline freely.
        jt = big.tile([FJ, JC], bf16, tag="jt")
        for c0 in range(0, JC, 2048):
            w = min(2048, JC - c0)
            nc.sync.dma_start(jt[:, c0 : c0 + w],
                              jets_d.ap()[:, c0 : c0 + w])
        x1 = big.tile([H, JC], bf16, tag="x1")
        x2 = big.tile([H, JC], bf16, tag="x2")
        x = big.tile([H, JC], bf16, tag="x")
        z = big.tile([H, JC], bf16, tag="z")
        plan = [(x1, w1t[:], jt, 0), (x2, w2t, x1, 1),
                (x, w3t, x2, 2), (z, wzt, x, None)]
        for li, (dst, wt, src, bias_col) in enumerate(plan):
            for ci, c0 in enumerate(range(0, JC, 1024)):
                w = min(1024, JC - c0)
                ps = mm.tile([H, 1024], f32, tag="mm")
                for s0 in range(0, w, 512):
                    sw = min(512, w - s0)
                    nc.tensor.matmul(ps[:, s0 : s0 + sw], wt,
                                     src[:, c0 + s0 : c0 + s0 + sw],
                                     start=True, stop=True)
                # Split each layer's evacs between Vector (front half) and
                # Scalar (back half): layers are sequential, so a
                # per-layer split would idle one engine per layer; V gets
                # the front so it finishes early and rolls into y1 (whose
                # first chunks need the first z columns).
                nchunks = -(-JC // 1024)
                on_v = ci < nchunks * 6 // 13
                if bias_col is None:
                    if on_v:
                        nc.vector.tensor_copy(dst[:, c0 : c0 + w],
                                              ps[:, :w])
                    else:
                        nc.scalar.copy(dst[:, c0 : c0 + w], ps[:, :w])
                elif on_v:
                    nc.vector.tensor_scalar(
                        dst[:, c0 : c0 + w], ps[:, :w],
                        bv[:, bias_col : bias_col + 1], 0.0,
                        ALU.add, ALU.max)
                else:
                    nc.scalar.activation(
                        dst[:, c0 : c0 + w], ps[:, :w], AF.Relu,
                        bias=bv[:, bias_col : bias_col + 1])
                if li == 2:
                    nc.sync.dma_start(xout_d.ap()[:, c0 : c0 + w],
                                      dst[:, c0 : c0 + w])

        # ---- phase 2: all pair chunks stream through y1 -> y2 -> y3 ->
        # DRAM. Every chunk's z is ready, so chunks are fully independent
        # and pipeline across V (y1), PE (matmuls), S/V (evacs), DMA.
        # y3 matmuls run one chunk behind y2 (software pipeline) so PE
        # never waits on the y2 evac of the chunk it just produced.
        yout_off = 0
        y3_i = [0]
        pend = [None]  # (y2_tile, w, yout_off)

        def emit_y3(y2, w, yoff):
            y3 = scr.tile([H, 4096], bf16, tag="y3")
            for n0 in range(0, w, 1024):
                cw = min(1024, w - n0)
                ps = mm.tile([H, 1024], f32, tag="mm")
                for s0 in range(0, cw, 512):
                    sw = min(512, cw - s0)
                    nc.tensor.matmul(ps[:, s0 : s0 + sw], w5t,
                                     y2[:, n0 + s0 : n0 + s0 + sw],
                                     start=True, stop=True)
                dst = y3[:, n0 : n0 + cw]
                if Y3_PAT[y3_i[0] % len(Y3_PAT)] == "v":
                    nc.vector.tensor_scalar(dst, ps[:, :cw], bv[:, 5:6],
                                            0.0, ALU.add, ALU.max)
                else:
                    nc.scalar.activation(dst, ps[:, :cw], AF.Relu,
                                         bias=bv[:, 5:6])
                y3_i[0] += 1
            nc.sync.dma_start(yout_d.ap()[:, yoff : yoff + w], y3[:, :w])

        jets_off = 0
        for g, cap in groups:
            assert cap % 8 == 0 and cap <= 512
            prs = pairs_of(g)
            PG = len(prs)
            SC = max(1, 4096 // cap)
            for p0 in range(0, PG, SC):
                k = min(SC, PG - p0)
                w = k * cap
                y1 = scr.tile([H, 4096], bf16, tag="y1")
                s = 0
                while s < k:
                    i = prs[p0 + s][0]
                    r = 1
                    while s + r < k and prs[p0 + s + r][0] == i:
                        r += 1
                    j0 = prs[p0 + s][1]
                    zb = jets_off
                    nc.vector.tensor_tensor(
                        r3(y1[:, s * cap : (s + r) * cap], r),
                        r3(z[:, zb + i * cap : zb + (i + 1) * cap],
                           1).broadcast_to([H, r, cap]),
                        r3(z[:, zb + j0 * cap : zb + (j0 + r) * cap], r),
                        ALU.add)
                    s += r
                nc.vector.tensor_scalar(y1[:, :w], y1[:, :w], bv[:, 3:4],
                                        0.0, ALU.add, ALU.max)
                y2 = scr.tile([H, 4096], bf16, tag="y2")
                for n0 in range(0, w, 1024):
                    cw = min(1024, w - n0)
                    ps = mm.tile([H, 1024], f32, tag="mm")
                    for s0 in range(0, cw, 512):
                        sw = min(512, cw - s0)
                        nc.tensor.matmul(ps[:, s0 : s0 + sw], w4t,
                                         y1[:, n0 + s0 : n0 + s0 + sw],
                                         start=True, stop=True)
                    nc.scalar.activation(y2[:, n0 : n0 + cw], ps[:, :cw],
                                         AF.Relu, bias=bv[:, 4:5])
                if pend[0] is not None:
                    emit_y3(*pend[0])
                pend[0] = (y2, w, yout_off)
                yout_off += w
            jets_off += g * cap
        emit_y3(*pend[0])

    nc.compile()
    return nc


# ---------------- host-side math ----------------

BN_EPS = 1e-3


def fold_params(inp):
    """Fold normalization + BN into per-layer (W, b). All numpy fp32."""
    mean_j = np.asarray(inp["mean_jets"], np.float32)
    std_j = np.asarray(inp["std_jets"], np.float32)
    w1f = np.asarray(inp["w1_first"], np.float32)
    w1r = np.asarray(inp["w1_rest"], np.float32)
    bn1 = np.asarray(inp["bn1"], np.float32)  # [3,4,H]: gamma, beta, mean, var
    w2f = np.asarray(inp["w2_first"], np.float32)
    w2r = np.asarray(inp["w2_rest"], np.float32)
    bn2 = np.asarray(inp["bn2"], np.float32)

    def bn_sb(row):
        gm, bt, mu, vv = row[0], row[1], row[2], row[3]
        s = gm / np.sqrt(vv + BN_EPS)
        return s.astype(np.float32), (bt - mu * s).astype(np.float32)

    s11, t11 = bn_sb(bn1[0]); s12, t12 = bn_sb(bn1[1]); s13, t13 = bn_sb(bn1[2])
    s21, t21 = bn_sb(bn2[0]); s22, t22 = bn_sb(bn2[1]); s23, t23 = bn_sb(bn2[2])

    A = w1f / std_j[:, None]
    c = -(mean_j / std_j) @ w1f
    return dict(
        W1=A * s11[None, :], b1=c * s11 + t11,
        W2=w1r[0] * s12[None, :], b2=t12,
        W3=w1r[1] * s13[None, :], b3=t13,
        Wz=w2f * s21[None, :], bz=t21,
        W4=w2r[0] * s22[None, :], b4=t22,
        W5=w2r[1] * s23[None, :], b5=t23,
    )


# ---------------- full kernel entry point ----------------

N_CORES = 8

_cache = {}
_TRACE = [False]
_LAST_RESULT = [None]


def _get_program(groups_key):
    if groups_key not in _cache:
        _cache[groups_key] = build_program(list(groups_key))
    return _cache[groups_key]


def _np_dt(dt):
    return mybir.dt.np(dt)


def _plan(n):
    """Returns (groups, slots): groups = [(g, cap)], slots[c][gi] =
    (padded index array, real count) for core c, group gi."""
    gs = []
    idx_by_g = {}
    for g in range(2, 11):
        idx = np.nonzero(n == g)[0]
        if len(idx):
            gs.append(g)
            idx_by_g[g] = idx
    stray = np.nonzero((n < 2) | (n > 10))[0]
    if len(stray):
        if not gs:
            gs.append(2)
            idx_by_g[2] = stray
        else:
            idx_by_g[gs[-1]] = np.concatenate([idx_by_g[gs[-1]], stray])
    # Descending size order: each group's jets chain is emitted inside
    # the previous (bigger) group's pair phase, so it pipelines fully.
    gs = sorted(gs, key=lambda g: -g)
    groups = []
    slots = [[] for _ in range(N_CORES)]
    for g in gs:
        idx = idx_by_g[g]
        per_core = [idx[c::N_CORES] for c in range(N_CORES)]
        mx = max(len(p) for p in per_core)
        cap = max(8, ((mx + 7) // 8) * 8)
        groups.append((g, cap))
        fill = idx[0]
        for c in range(N_CORES):
            p = per_core[c]
            pad = np.full(cap, p[0] if len(p) else fill, dtype=np.int64)
            pad[: len(p)] = p
            slots[c].append((pad, len(p)))
    return groups, slots


def _pack_jets(jets, groups, slots_c):
    cols = []
    for (g, cap), (ids, _cnt) in zip(groups, slots_c):
        ev = jets[ids][:, :g, :]  # [cap, g, 16]
        cols.append(np.ascontiguousarray(ev.transpose(2, 1, 0)).reshape(
            FJ, g * cap))
    return np.concatenate(cols, axis=1).astype(_np_dt(bf16), copy=False)


def kernel(**inputs):
    from concourse.bass_utils import run_bass_kernel_spmd

    jets = np.asarray(inputs["inputs_jets"], dtype=np.float32)
    B = jets.shape[0]
    mask = (jets != 0.0).any(-1)
    n = mask.sum(-1).astype(np.int64)
    # compact valid jets to the front (no-op for the standard generator)
    if not np.array_equal(mask, np.arange(jets.shape[1])[None, :] < n[:, None]):
        order = np.argsort(~mask, axis=1, kind="stable")
        jets = np.take_along_axis(jets, order[:, :, None], axis=1)

    P = fold_params(inputs)
    groups, slots = _plan(n)
    nc = _get_program(tuple(groups))

    bvec = np.zeros((H, 8), np.float32)
    for i, k in enumerate(["b1", "b2", "b3", "bz", "b4", "b5"]):
        bvec[:, i] = P[k]
    bnp = _np_dt(bf16)
    wall = np.concatenate(
        [P["W2"], P["W3"], P["Wz"], P["W4"], P["W5"]], axis=1)
    common = {
        "w1": P["W1"].astype(bnp), "wall": wall.astype(bnp),
        "bvec": bvec,
    }
    in_maps = []
    for c in range(N_CORES):
        m = dict(common)
        m["jets"] = _pack_jets(jets, groups, slots[c])
        in_maps.append(m)

    res = run_bass_kernel_spmd(nc, in_maps, core_ids=list(range(N_CORES)),
                               trace=_TRACE[0])
    _LAST_RESULT[0] = res

    agg_x = np.empty((B, 4 * H), np.float32)
    agg_y = np.empty((B, 4 * H), np.float32)
    for c in range(N_CORES):
        ox = np.asarray(res.results[c]["xout"])  # [H, JC] bf16
        oy = np.asarray(res.results[c]["yout"])  # [H, YC] bf16
        joff = 0
        yoff = 0
        for (g, cap), (ids, cnt) in zip(groups, slots[c]):
            PGg = g * (g - 1) // 2
            ii = ids[:cnt]
            xb = ox[:, joff : joff + g * cap].astype(np.float32)
            xb = xb.reshape(H, g, cap)[:, :, :cnt]
            sx = xb.sum(1).T
            mx = xb.max(1).T
            qx = (xb * xb).sum(1).T
            mean_x = sx / g
            agg_x[ii] = np.concatenate(
                [sx, mx, mean_x, qx / g - mean_x * mean_x], axis=1)
            yb = oy[:, yoff : yoff + PGg * cap].astype(np.float32)
            yb = yb.reshape(H, PGg, cap)[:, :, :cnt]
            sy = yb.sum(1).T
            my = yb.max(1).T
            qy = (yb * yb).sum(1).T
            mean_y = sy / PGg
            agg_y[ii] = np.concatenate(
                [sy, my, mean_y, qy / PGg - mean_y * mean_y], axis=1)
            joff += g * cap
            yoff += PGg * cap
    return agg_x, agg_y
